# revision 3
# baseline (speedup 1.0000x reference)
"""Trainium2 Bass kernel for nn_BinaryDiff.

Reference computes:
    out = x @ base_T + coeff * (x @ signs),  signs = 2*mask_bits - 1
which algebraically equals a single dense matmul:
    out = x @ W,  W = base_T + coeff * (2*mask_bits - 1)

v2 strategy (vs f32r baseline at ~509us):
  - bf16 operands: same 1 cycle/row PE stream rate as f32r, but enables
    the compiler's Fast Weight Load path (LDWEIGHTS 2x faster, off for
    fp32/f32r) and halves HBM traffic. Precision: |err| ~ 0.01 absolute
    vs a 0.15 tolerance budget.
  - Transposed orientation: compute outT = W.T @ xT with the W tile as
    the PE's stationary operand and the resident xT panel as the moving
    operand. Each 128x128 W tile is loaded into the array ONCE (1024
    LDWEIGHTS total instead of 2048) and streams the full 1024-row x
    panel through it (one compound matmul -> LDWEIGHTS + 2x512 MATMUL).
  - W is packed on host per n-group so each group's weights arrive in
    one contiguous 2MB DMA (16KB/partition lines).
  - PSUM: 2 live n-chunks x [128,1024]f32 (2 banks each), double
    buffered = all 8 banks; drains fully overlap the next group's MMs.
  - Host folds W, pre-transposes x (K-major), and transposes the
    per-core outT back - no on-device transposes.

Shard: rows of x (M = B*S = 8192) across 8 cores, 1024 rows each; W
replicated.
"""

import numpy as np
import ml_dtypes

B, S, DIN, DOUT = 4, 2048, 4096, 4096
NCORES = 8
MTOT = B * S
MSHARD = MTOT // NCORES  # 1024

KT = DIN // 128          # 32 k-tiles
NGROUPS = 16             # n-groups of 256 cols (2 chunks of 128)
GCOLS = DOUT // NGROUPS  # 256

_CACHE = {}


def _build(compound=True):
    import concourse.bacc as bacc
    import concourse.mybir as mybir
    import concourse.tile as tile

    f32 = mybir.dt.float32
    bf16 = mybir.dt.bfloat16

    nc = bacc.Bacc()
    xt = nc.declare_dram_parameter("xt", [DIN, MSHARD], bf16, isOutput=False)
    w = nc.declare_dram_parameter("w", [NGROUPS, 128, KT * GCOLS], bf16, isOutput=False)
    outT = nc.declare_dram_parameter("outT", [DOUT, MSHARD], f32, isOutput=True)

    with tile.TileContext(nc) as tc:
        with (
            tc.tile_pool(name="xt_pool", bufs=1) as xt_pool,
            tc.tile_pool(name="w_pool", bufs=2) as w_pool,
            tc.tile_pool(name="ps_pool", bufs=2, space="PSUM") as ps_pool,
            tc.tile_pool(name="o_pool", bufs=4) as o_pool,
        ):
            # Resident x^T panel, one tile per k so dependency tracking is
            # per-k and compute starts as soon as the first k-tile lands.
            xts = [
                xt_pool.tile([128, MSHARD], bf16, tag=f"xt{k}", name=f"xt{k}")
                for k in range(KT)
            ]
            # First k-tiles ride the HWDGE sync ring (fast first byte);
            # the rest ride gpsimd so they don't queue behind W loads.
            for k in range(KT):
                eng = nc.sync if k < 2 else nc.gpsimd
                eng.dma_start(xts[k][:], xt[k * 128:(k + 1) * 128, :])

            wgs = []
            for g in range(NGROUPS):
                w_t = w_pool.tile([128, KT * GCOLS], bf16, tag="w", name=f"w_{g}")
                if g == 0:
                    # Split group-0's load so k=0 weights land in ~1us and
                    # the PE starts while the rest streams in.
                    nc.sync.dma_start(w_t[:, :GCOLS], w[0, :, :GCOLS])
                    nc.sync.dma_start(w_t[:, GCOLS:8 * GCOLS], w[0, :, GCOLS:8 * GCOLS])
                    nc.sync.dma_start(w_t[:, 8 * GCOLS:], w[0, :, 8 * GCOLS:])
                else:
                    nc.sync.dma_start(w_t[:], w[g])
                wgs.append(w_t)

            for g in range(NGROUPS):
                w_t = wgs[g]
                ps = [
                    ps_pool.tile([128, MSHARD], f32, tag=f"ps{i}", name=f"ps{i}_{g}")
                    for i in range(2)
                ]
                for k in range(KT):
                    for i in range(2):
                        lhsT = w_t[:, k * GCOLS + i * 128: k * GCOLS + (i + 1) * 128]
                        if compound:
                            nc.tensor.matmul(
                                ps[i][:], lhsT, xts[k][:],
                                start=(k == 0), stop=(k == KT - 1),
                            )
                        else:
                            for h in range(2):
                                nc.tensor.matmul(
                                    ps[i][:, h * 512:(h + 1) * 512],
                                    lhsT,
                                    xts[k][:, h * 512:(h + 1) * 512],
                                    start=(k == 0), stop=(k == KT - 1),
                                )
                for i in range(2):
                    o_t = o_pool.tile([128, MSHARD], f32, tag="o", name=f"o_{g}_{i}")
                    nc.vector.tensor_copy(o_t[:], ps[i][:])
                    n0 = g * GCOLS + i * 128
                    nc.scalar.dma_start(outT[n0:n0 + 128, :], o_t[:])

    nc.finalize()
    return nc


def _get_nc():
    if "nc" not in _CACHE:
        _CACHE["nc"] = _build(compound=False)
    return _CACHE["nc"]


def _run(x, base_T, mask_bits, coeff, trace=False):
    from concourse.bass_utils import run_bass_kernel_spmd

    nc = _get_nc()

    W = (np.asarray(base_T, dtype=np.float32)
         + np.float32(coeff[0]) * (2.0 * np.asarray(mask_bits, dtype=np.float32) - 1.0))
    Wb = W.astype(ml_dtypes.bfloat16)
    # Pack per n-group: WP[g, p, k*256+c] = W[k*128+p, g*256+c]
    WP = np.ascontiguousarray(
        Wb.reshape(KT, 128, NGROUPS, GCOLS).transpose(2, 1, 0, 3)
    ).reshape(NGROUPS, 128, KT * GCOLS)

    X = np.asarray(x, dtype=np.float32).reshape(MTOT, DIN)

    in_maps = []
    for c in range(NCORES):
        xt_c = np.ascontiguousarray(
            X[c * MSHARD:(c + 1) * MSHARD, :].T.astype(ml_dtypes.bfloat16)
        )
        in_maps.append({"xt": xt_c, "w": WP})

    res = run_bass_kernel_spmd(nc, in_maps, list(range(NCORES)), trace=trace)
    outs = [
        np.ascontiguousarray(res.results[c]["outT"].T) for c in range(NCORES)
    ]
    full = np.concatenate(outs, axis=0).reshape(B, S, DOUT).astype(np.float32)
    return full, res


def kernel(x, base_T, mask_bits, coeff):
    full, _ = _run(x, base_T, mask_bits, coeff, trace=False)
    return full
